# revision 11
# baseline (speedup 1.0000x reference)
"""AttentionKVCache Trainium2 kernel — data-parallel over batch (8 cores).

Per core (one batch element, all weights replicated):
  q = x @ wq^T ; k_new = x @ wk^T ; v_new = x @ wv^T            (bf16 matmuls)
  k = [k_cache; k_new] ; v = [v_cache; v_new]
  scores = (q @ k^T) * scale ; attn = softmax(scores)
  out = attn @ v ; y = out @ wo^T

TensorE contracts over the partition dim of both operands, so every operand
of a matmul needs its contraction index on partitions. Natural DMA layouts
give x:[s,d], w:[e,d], caches:[kv,e] — weights and k_cache must be
transposed on-chip. All transposes are PE matmuls against a bf16 identity
(regular matmul, not transpose-mode): psum = tile.T @ I, then a DVE/ACT
copy-cast psum(f32) -> sbuf(bf16). DMA-xbar transpose is unusable here
(DmaTransposeAnt carries at most one semaphore wait; Tile needs more).

  xT  [d, s]    PE-transposed from cast-loaded x
  wT  [d, e]    per 512-row weight slice: 64 PE transposes -> wT tile
  qT  [e, s]    direct matmul (lhsT = wT tile, rhs = xT tile), N=128
  knT [e, s]    same
  vn  [s, e]    natural (lhsT = xT, rhs = wvT slices), N=512
  scores[s,kv]  lhsT = qT tile, rhs = kT tile (k_cache PE-transposed)
  attnT [kv,s]  PE transpose of exp() chunks
  out  [s, e]   natural: lhsT = attnT tile, rhs = v natural, N=512
  outT [e, s]   PE transpose
  y  [s, dout]  lhsT = outT tile, rhs = woT; row-scaled by 1/l at copy-out
"""

import math

import numpy as np

import concourse.bass as bass
import concourse.mybir as mybir
import concourse.tile as tile
from concourse.bass_utils import run_bass_kernel_spmd
from concourse.masks import make_identity

B = 8
Q = 128          # query length
KV = 4096        # cache length
D = 2048         # model dim
KVT = KV + Q     # 4224 total keys
NT = D // 128    # 16 tiles along d/e
NKVT = KVT // 128  # 33 tiles along kv
SCALE = 1.0 / math.sqrt(D)

F32 = mybir.dt.float32
BF16 = mybir.dt.bfloat16

Exp = mybir.ActivationFunctionType.Exp
Copy = mybir.ActivationFunctionType.Copy
AX = mybir.AxisListType.X


def build_nc() -> bass.Bass:
    nc = bass.Bass()

    x_d = nc.declare_dram_parameter("x", [Q, D], F32, isOutput=False)
    kc_d = nc.declare_dram_parameter("k_cache", [KV, D], F32, isOutput=False)
    vc_d = nc.declare_dram_parameter("v_cache", [KV, D], F32, isOutput=False)
    wq_d = nc.declare_dram_parameter("wq", [D, D], F32, isOutput=False)
    wk_d = nc.declare_dram_parameter("wk", [D, D], F32, isOutput=False)
    wv_d = nc.declare_dram_parameter("wv", [D, D], F32, isOutput=False)
    wo_d = nc.declare_dram_parameter("wo", [D, D], F32, isOutput=False)
    y_d = nc.declare_dram_parameter("out", [Q, D], F32, isOutput=True)

    with tile.TileContext(nc) as tc:
        with (
            tc.tile_pool(name="res", bufs=1) as res,
            tc.tile_pool(name="nat", bufs=3) as natp,
            tc.tile_pool(name="T", bufs=2) as Tp,
            tc.tile_pool(name="achunk", bufs=3) as achunk,
            tc.tile_pool(name="small", bufs=2) as small,
            tc.tile_pool(name="bank", bufs=8, space="PSUM") as bankp,
        ):
            ident = res.tile([128, 128], BF16)
            make_identity(nc, ident)

            copy_flip = [0]
            psn = [0]

            def copy_cast(dst_ap, src_ap):
                """PSUM->SBUF copy (casts via dst dtype); alternate DVE/ACT."""
                if copy_flip[0] % 2 == 0:
                    nc.vector.tensor_copy(out=dst_ap, in_=src_ap)
                else:
                    nc.scalar.copy(out=dst_ap, in_=src_ap)
                copy_flip[0] += 1

            def bank_tile():
                psn[0] += 1
                return bankp.tile([128, 512], F32, tag="bank", name=f"pb{psn[0]}")

            def pe_transpose_block(src_tiles, dst_block_fn):
                """Transpose a list of [128,128] bf16 SBUF APs via PE matmul
                with identity. Batches 4 per PSUM bank; dst_block_fn(b0, nb)
                returns ONE contiguous [128, nb*128] SBUF AP covering the
                destinations of transposed tiles b0..b0+nb-1."""
                n = len(src_tiles)
                for b0 in range(0, n, 4):
                    nb = min(4, n - b0)
                    ps = bank_tile()
                    for i in range(nb):
                        nc.tensor.matmul(
                            ps[:, i * 128 : (i + 1) * 128],
                            lhsT=src_tiles[b0 + i],
                            rhs=ident,
                            start=(i == 0),
                            stop=(i == nb - 1),
                        )
                    copy_cast(dst_block_fn(b0, nb), ps[:, : nb * 128])

            # ---------------- x: load + transpose ----------------
            x_bf = res.tile([128, D], BF16)
            nc.gpsimd.dma_start(out=x_bf, in_=x_d[:, :])  # f32->bf16 cast load
            xT = res.tile([128, NT, 128], BF16)  # xT[p, t, s] = x[s, t*128+p]
            pe_transpose_block(
                [x_bf[:, t * 128 : (t + 1) * 128] for t in range(NT)],
                lambda b0, nb: xT[:, b0 : b0 + nb, :].rearrange("p t s -> p (t s)"),
            )

            def load_cast_512(dram, r0, nm):
                """One SWDGE cast-load of 512 rows: tile[p, c, :] =
                dram[r0 + c*128 + p, :] in bf16."""
                t = natp.tile([128, 4, D], BF16, tag="nat", name=nm)
                nc.gpsimd.dma_start(
                    out=t,
                    in_=dram[r0 : r0 + 512, :].rearrange("(c p) d -> p c d", p=128),
                )
                return t

            def transpose_512(nat, nm):
                """wT[p, c, t, j] = nat[j, c, t*128+p] (= w[r0+c*128+j, t*128+p]):
                [d, e]-layout tiles for one 512-row slice of a weight."""
                wT = Tp.tile([128, 4, NT, 128], BF16, tag="T", name=nm)
                for c in range(4):
                    pe_transpose_block(
                        [nat[:, c, t * 128 : (t + 1) * 128] for t in range(NT)],
                        lambda b0, nb, c=c: wT[:, c, b0 : b0 + nb, :].rearrange(
                            "p t j -> p (t j)"
                        ),
                    )
                return wT

            # ---------------- projections ----------------
            qT = res.tile([128, NT, 128], BF16)   # [e, s]
            knT = res.tile([128, NT, 128], BF16)  # [e, s]
            vn = res.tile([128, D], BF16)         # [s, e] natural

            for iw, (w_dram, mode) in enumerate(
                ((wq_d, "qT"), (wk_d, "kT"), (wv_d, "nat"))
            ):
                for es in range(4):
                    nat = load_cast_512(w_dram, es * 512, f"w{iw}n{es}")
                    wT = transpose_512(nat, f"w{iw}T{es}")
                    ps = bank_tile()
                    if mode == "nat":
                        for dt in range(NT):
                            nc.tensor.matmul(
                                ps,
                                lhsT=xT[:, dt, :],
                                rhs=wT[:, :, dt, :],
                                start=(dt == 0),
                                stop=(dt == NT - 1),
                            )
                        copy_cast(vn[:, es * 512 : (es + 1) * 512], ps)
                    else:
                        # direct transposed projection: [e-chunk, s] slices
                        dest = qT if mode == "qT" else knT
                        for c in range(4):
                            for dt in range(NT):
                                nc.tensor.matmul(
                                    ps[:, c * 128 : (c + 1) * 128],
                                    lhsT=wT[:, c, dt, :],
                                    rhs=xT[:, dt, :],
                                    start=(c == 0 and dt == 0),
                                    stop=(c == 3 and dt == NT - 1),
                                )
                        copy_cast(
                            dest[:, es * 4 : (es + 1) * 4, :].rearrange(
                                "p c s -> p (c s)"
                            ),
                            ps,
                        )

            # ---------------- scores = q @ k^T ----------------
            scores = res.tile([128, KVT], F32)
            for ck in range(8):  # 8 chunks of 512 cache positions
                nat = load_cast_512(kc_d, ck * 512, f"kn{ck}")
                kT = transpose_512(nat, f"kT{ck}")
                ps = bank_tile()
                for et in range(NT):
                    nc.tensor.matmul(
                        ps,
                        lhsT=qT[:, et, :],
                        rhs=kT[:, :, et, :],
                        start=(et == 0),
                        stop=(et == NT - 1),
                    )
                copy_cast(scores[:, ck * 512 : (ck + 1) * 512], ps)
            # new-token scores (kv = 4096..4223)
            ps = bank_tile()
            for et in range(NT):
                nc.tensor.matmul(
                    ps[:, :128],
                    lhsT=qT[:, et, :],
                    rhs=knT[:, et, :],
                    start=(et == 0),
                    stop=(et == NT - 1),
                )
            copy_cast(scores[:, KV:KVT], ps[:, :128])

            # ---------------- softmax ----------------
            negm = small.tile([128, 1], F32)
            nc.vector.reduce_max(out=negm, in_=scores, axis=AX, negate=True)
            negms = small.tile([128, 1], F32)
            nc.vector.tensor_scalar_mul(negms, negm, SCALE)  # -max * scale

            attnT = res.tile([128, NKVT, 128], BF16)  # [kv, s]
            lparts = small.tile([128, 9], F32)
            for ck in range(9):
                w = 512 if ck < 8 else 128
                ac = achunk.tile([128, 512], BF16, tag="ac", name=f"ac{ck}")
                nc.scalar.activation(
                    out=ac[:, :w],
                    in_=scores[:, ck * 512 : ck * 512 + w],
                    func=Exp,
                    bias=negms,
                    scale=SCALE,
                    accum_out=lparts[:, ck : ck + 1],
                )
                pe_transpose_block(
                    [ac[:, t * 128 : (t + 1) * 128] for t in range(w // 128)],
                    lambda b0, nb, ck=ck: attnT[
                        :, ck * 4 + b0 : ck * 4 + b0 + nb, :
                    ].rearrange("p t s -> p (t s)"),
                )
            lsum = small.tile([128, 1], F32)
            nc.vector.reduce_sum(out=lsum, in_=lparts, axis=AX)
            rinv = small.tile([128, 1], F32)
            nc.vector.reciprocal(out=rinv, in_=lsum)

            # ------------- out = attn @ v  (natural, unnormalized) -------
            ps_out = []
            for _i in range(4):
                ps_out.append(
                    bankp.tile([128, 512], F32, tag="bank", name=f"psout{_i}")
                )
            for ck in range(8):  # v cache chunks of 512 rows
                vt = load_cast_512(vc_d, ck * 512, f"vld{ck}")
                for c in range(4):
                    kvt = ck * 4 + c
                    for ec in range(4):
                        nc.tensor.matmul(
                            ps_out[ec],
                            lhsT=attnT[:, kvt, :],
                            rhs=vt[:, c, ec * 512 : (ec + 1) * 512],
                            start=(kvt == 0),
                            stop=False,
                        )
            # v_new tail
            for ec in range(4):
                nc.tensor.matmul(
                    ps_out[ec],
                    lhsT=attnT[:, NKVT - 1, :],
                    rhs=vn[:, ec * 512 : (ec + 1) * 512],
                    start=False,
                    stop=True,
                )
            out_bf = res.tile([128, D], BF16)
            for ec in range(4):
                copy_cast(out_bf[:, ec * 512 : (ec + 1) * 512], ps_out[ec])
            outT = res.tile([128, NT, 128], BF16)
            pe_transpose_block(
                [out_bf[:, t * 128 : (t + 1) * 128] for t in range(NT)],
                lambda b0, nb: outT[:, b0 : b0 + nb, :].rearrange("p t s -> p (t s)"),
            )

            # ---------------- y = out @ wo^T, row-scaled by 1/l ----------
            y_sb = res.tile([128, D], F32)
            for ds_ in range(4):
                nat = load_cast_512(wo_d, ds_ * 512, f"won{ds_}")
                woT = transpose_512(nat, f"woT{ds_}")
                ps = bank_tile()
                for et in range(NT):
                    nc.tensor.matmul(
                        ps,
                        lhsT=outT[:, et, :],
                        rhs=woT[:, :, et, :],
                        start=(et == 0),
                        stop=(et == NT - 1),
                    )
                nc.scalar.activation(
                    out=y_sb[:, ds_ * 512 : (ds_ + 1) * 512],
                    in_=ps,
                    func=Copy,
                    scale=rinv,
                )
                nc.sync.dma_start(
                    out=y_d[:, ds_ * 512 : (ds_ + 1) * 512],
                    in_=y_sb[:, ds_ * 512 : (ds_ + 1) * 512],
                )

    return nc


# ---------------------------------------------------------------------------
# Host-side wrapper
# ---------------------------------------------------------------------------

_NC = None


def _get_nc():
    global _NC
    if _NC is None:
        _NC = build_nc()
    return _NC


def _make_in_maps(x, k_cache, v_cache, wq, wk, wv, wo):
    f = lambda a: np.ascontiguousarray(np.asarray(a), dtype=np.float32)
    x, k_cache, v_cache = f(x), f(k_cache), f(v_cache)
    wq, wk, wv, wo = f(wq), f(wk), f(wv), f(wo)
    maps = []
    for b in range(B):
        maps.append(
            {
                "x": x[b],
                "k_cache": k_cache[b],
                "v_cache": v_cache[b],
                "wq": wq,
                "wk": wk,
                "wv": wv,
                "wo": wo,
            }
        )
    return maps


def _install_hookfix():
    """The stock concourse neuronx_cc hook asserts a single HLO computation;
    this jax version lowers shard_map as `call`-indirected HLO. Patch the
    validation (operand order must still chain to entry params in order)."""
    import base64
    import tempfile

    import orjson

    from concourse import bass2jax
    from concourse.bass2jax import (
        _decompress_ant_bir,
        rename_neff_tensors_and_patch_header,
        compile_bir_kernel,
    )

    def _trace_to_param(id2ins, ins):
        seen = 0
        while ins.opcode != "parameter":
            if len(ins.operand_ids) != 1 or seen > 20:
                return None
            ins = id2ins[ins.operand_ids[0]]
            seen += 1
        return ins.parameter_number

    def _split_multi_waits(bir_bytes):
        """This walrus build accepts at most one semaphore wait (and update)
        per instruction. Move extra waits onto prepended single-wait
        EventSemaphore instructions on the same engine (engine streams are
        serial, so a preceding wait is equivalent); extra updates onto
        appended EventSemaphores."""
        bir = orjson.loads(bir_bytes)
        ctr = [0]
        for fn in bir.get("functions", []):
            for blk in fn.get("blocks", []):
                newins = []
                changed = False
                for ins in blk.get("instructions", []):
                    si = ins.get("sync_info")
                    eng = ins.get("engine")
                    waits = (si or {}).get("on_wait") or []
                    if si is not None and eng is not None and len(waits) > 1:
                        for w in waits[:-1]:
                            ctr[0] += 1
                            newins.append(
                                {
                                    "debug": ins.get("debug", 0),
                                    "engine": eng,
                                    "ins": [],
                                    "outs": [],
                                    "name": f"evw{ctr[0]}_{ins['name']}",
                                    "opcode": "EventSemaphore",
                                    "sync_info": {"on_update": [], "on_wait": [w]},
                                }
                            )
                        si["on_wait"] = [waits[-1]]
                        changed = True
                    newins.append(ins)
                    ups = (si or {}).get("on_update") or []
                    if si is not None and eng is not None and len(ups) > 1:
                        for u in ups[1:]:
                            ctr[0] += 1
                            newins.append(
                                {
                                    "debug": ins.get("debug", 0),
                                    "engine": eng,
                                    "ins": [],
                                    "outs": [],
                                    "name": f"evu{ctr[0]}_{ins['name']}",
                                    "opcode": "EventSemaphore",
                                    "sync_info": {"on_update": [u], "on_wait": []},
                                }
                            )
                        si["on_update"] = ups[:1]
                        changed = True
                if changed:
                    blk["instructions"] = newins
        return orjson.dumps(bir)

    def neuronx_cc_hook(code, code_format, platform_version, file_prefix):
        import libneuronxla
        import libneuronxla.proto.hlo_pb2
        from libneuronxla.libncc import _wrap_neff_as_custom_call

        assert libneuronxla.orig_neuronx_cc is not None

        def _orig():
            return libneuronxla.orig_neuronx_cc(
                code, code_format, platform_version, file_prefix
            )

        if b"bass_exec" not in code:
            return _orig()
        assert code_format.decode() == "hlo"
        code_proto = libneuronxla.proto.hlo_pb2.HloModuleProto.FromString(code)
        bass_exec_call = None
        body = None
        for comp in code_proto.computations:
            for ins in comp.instructions:
                if (
                    ins.opcode == "custom-call"
                    and ins.custom_call_target == "bass_exec"
                ):
                    assert bass_exec_call is None
                    bass_exec_call = ins
                    body = comp
        if bass_exec_call is None:
            return _orig()

        id2ins = {i.id: i for i in body.instructions}
        nums = [
            _trace_to_param(id2ins, id2ins[op_id])
            for op_id in bass_exec_call.operand_ids[:-1]
        ]
        if nums != list(range(len(nums))):
            raise ValueError(f"bass_exec operands not in param order: {nums}")

        entry = [
            c
            for c in code_proto.computations
            if c.id == code_proto.entry_computation_id
        ]
        entry = entry[0] if entry else code_proto.computations[-1]
        if entry.id != body.id:
            eid2ins = {i.id: i for i in entry.instructions}
            call_ins = [
                i
                for i in entry.instructions
                if i.opcode == "call"
                and list(i.called_computation_ids) == [body.id]
            ]
            assert len(call_ins) == 1
            enums = [
                _trace_to_param(eid2ins, eid2ins[op_id])
                for op_id in call_ins[0].operand_ids
            ]
            if enums != list(range(len(enums))):
                raise ValueError(f"shmap call operands not in param order: {enums}")

        config = orjson.loads(base64.standard_b64decode(bass_exec_call.backend_config))
        assert len(config["in_names"]) == len(bass_exec_call.operand_ids)
        in_rename = {name: f"input{i}" for i, name in enumerate(config["in_names"])}
        out_rename = {name: f"output{i}" for i, name in enumerate(config["out_names"])}
        neff_name = f"model_{code_proto.name.replace('/', '_')}.neff"
        ant_bir_str = _decompress_ant_bir(config["ant_bir"])
        ant_bir_str = _split_multi_waits(ant_bir_str)
        compile_dir = tempfile.TemporaryDirectory(delete=False)
        with compile_dir as compile_dir_path:
            neff_file = compile_bir_kernel(
                ant_bir_str, compile_dir_path, neff_name=neff_name
            )
            neff_data = rename_neff_tensors_and_patch_header(
                neff_file, in_rename | out_rename
            )
        compile_dir.cleanup()
        return 0, _wrap_neff_as_custom_call(code, neff_data)

    bass2jax.neuronx_cc_hook = neuronx_cc_hook
    try:
        import libneuronxla
    except ImportError:
        return
    if not hasattr(libneuronxla, "orig_neuronx_cc"):
        libneuronxla.orig_neuronx_cc = libneuronxla.neuronx_cc
    libneuronxla.neuronx_cc = neuronx_cc_hook


def run(inputs, trace=False, trace_cores=None):
    _install_hookfix()
    nc = _get_nc()
    maps = _make_in_maps(
        inputs["x"], inputs["k_cache"], inputs["v_cache"],
        inputs["wq"], inputs["wk"], inputs["wv"], inputs["wo"],
    )
    res = run_bass_kernel_spmd(
        nc,
        maps,
        core_ids=list(range(B)),
        trace=trace,
        trace_cores=trace_cores,
    )
    y = np.stack([res.results[b]["out"] for b in range(B)], axis=0)
    return y, res


def kernel(x, mask, k_cache, v_cache, wq, wk, wv, wo):
    y, _ = run(
        {
            "x": x,
            "k_cache": k_cache,
            "v_cache": v_cache,
            "wq": wq,
            "wk": wk,
            "wv": wv,
            "wo": wo,
        }
    )
    return y


# revision 12
# speedup vs baseline: 9.1814x; 9.1814x over previous
"""AttentionKVCache Trainium2 kernel — data-parallel over batch (8 cores).

Per core (one batch element, all weights replicated):
  q = x @ wq^T ; k_new = x @ wk^T ; v_new = x @ wv^T            (bf16 matmuls)
  k = [k_cache; k_new] ; v = [v_cache; v_new]
  scores = (q @ k^T) * scale ; attn = softmax(scores)
  out = attn @ v ; y = out @ wo^T

TensorE contracts over the partition dim of both operands, so every operand
of a matmul needs its contraction index on partitions. Natural DMA layouts
give x:[s,d], w:[e,d], caches:[kv,e] — weights and k_cache must be
transposed on-chip. All transposes are PE matmuls against a bf16 identity
(regular matmul: psum = tile.T @ I), then a DVE/ACT copy-cast
psum(f32) -> sbuf(bf16). DMA-xbar transpose is unusable here
(DmaTransposeAnt carries at most one semaphore wait; Tile needs more).

  xT  [d, s]    PE-transposed from cast-loaded x
  wT  [d, e]    per 512-row weight slice: 64 PE transposes -> wT tile
  qT  [e, s]    direct matmul (lhsT = wT tile, rhs = xT tile), N=128
  knT [e, s]    same
  vn  [s, e]    natural (lhsT = xT, rhs = wvT slices), N=512
  scores[s,kv]  lhsT = qT tile, rhs = kT tile (k_cache PE-transposed)
  attnT [kv,s]  PE transpose of exp() chunks
  out  [s, e]   natural: lhsT = attnT tile, rhs = v natural, N=512
  outT [e, s]   PE transpose
  y  [s, dout]  lhsT = outT tile, rhs = woT; row-scaled by 1/l at copy-out

build_nc(repeat=N) emits the whole computation N times (for slope-based
wall-clock timing that cancels the fixed per-dispatch overhead).
"""

import math

import numpy as np

import concourse.bass as bass
import concourse.mybir as mybir
import concourse.tile as tile
from concourse.bass_utils import run_bass_kernel_spmd
from concourse.masks import make_identity

B = 8
Q = 128          # query length
KV = 4096        # cache length
D = 2048         # model dim
KVT = KV + Q     # 4224 total keys
NT = D // 128    # 16 tiles along d/e
NKVT = KVT // 128  # 33 tiles along kv
SCALE = 1.0 / math.sqrt(D)

F32 = mybir.dt.float32
BF16 = mybir.dt.bfloat16

Exp = mybir.ActivationFunctionType.Exp
Copy = mybir.ActivationFunctionType.Copy
AX = mybir.AxisListType.X


def _emit_once(nc, pools, ident, state, rep, dram):
    res, natp, Tp, achunk, small, bankp = pools
    x_d, kc_d, vc_d, wq_d, wk_d, wv_d, wo_d, y_d = dram
    R = f"r{rep}"

    def copy_cast(dst_ap, src_ap):
        """PSUM->SBUF copy (casts via dst dtype); alternate DVE/ACT."""
        if state["flip"] % 2 == 0:
            nc.vector.tensor_copy(out=dst_ap, in_=src_ap)
        else:
            nc.scalar.copy(out=dst_ap, in_=src_ap)
        state["flip"] += 1

    def bank_tile():
        state["psn"] += 1
        return bankp.tile(
            [128, 512], F32, tag="bank", name=f"pb{state['psn']}{R}"
        )

    def pe_transpose_block(src_tiles, dst_block_fn):
        """Transpose a list of [128,128] bf16 SBUF APs via PE matmul with
        identity. Batches 4 per PSUM bank; dst_block_fn(b0, nb) returns ONE
        contiguous [128, nb*128] SBUF AP for transposed tiles b0..b0+nb-1."""
        n = len(src_tiles)
        for b0 in range(0, n, 4):
            nb = min(4, n - b0)
            ps = bank_tile()
            for i in range(nb):
                nc.tensor.matmul(
                    ps[:, i * 128 : (i + 1) * 128],
                    lhsT=src_tiles[b0 + i],
                    rhs=ident,
                    start=(i == 0),
                    stop=(i == nb - 1),
                )
            copy_cast(dst_block_fn(b0, nb), ps[:, : nb * 128])

    # ---------------- x: load + transpose ----------------
    x_bf = res.tile([128, D], BF16, tag="x_bf", name=f"x_bf{R}")
    nc.gpsimd.dma_start(out=x_bf, in_=x_d[:, :])  # f32->bf16 cast load
    xT = res.tile([128, NT, 128], BF16, tag="xT", name=f"xT{R}")
    pe_transpose_block(
        [x_bf[:, t * 128 : (t + 1) * 128] for t in range(NT)],
        lambda b0, nb: xT[:, b0 : b0 + nb, :].rearrange("p t s -> p (t s)"),
    )

    def load_cast_512(dram_t, r0, nm):
        """One SWDGE cast-load of 512 rows: tile[p, c, :] =
        dram[r0 + c*128 + p, :] in bf16."""
        t = natp.tile([128, 4, D], BF16, tag="nat", name=nm + R)
        nc.gpsimd.dma_start(
            out=t,
            in_=dram_t[r0 : r0 + 512, :].rearrange("(c p) d -> p c d", p=128),
        )
        return t

    def transpose_512(nat, nm):
        """wT[p, c, t, j] = nat[j, c, t*128+p] (= w[r0+c*128+j, t*128+p]):
        [d, e]-layout tiles for one 512-row slice of a weight."""
        wT = Tp.tile([128, 4, NT, 128], BF16, tag="T", name=nm + R)
        for c in range(4):
            pe_transpose_block(
                [nat[:, c, t * 128 : (t + 1) * 128] for t in range(NT)],
                lambda b0, nb, c=c: wT[:, c, b0 : b0 + nb, :].rearrange(
                    "p t j -> p (t j)"
                ),
            )
        return wT

    # ---------------- projections ----------------
    qT = res.tile([128, NT, 128], BF16, tag="qT", name=f"qT{R}")
    knT = res.tile([128, NT, 128], BF16, tag="knT", name=f"knT{R}")
    vn = res.tile([128, D], BF16, tag="vn", name=f"vn{R}")

    for iw, (w_dram, mode) in enumerate(
        ((wq_d, "qT"), (wk_d, "kT"), (wv_d, "nat"))
    ):
        for es in range(4):
            nat = load_cast_512(w_dram, es * 512, f"w{iw}n{es}")
            wT = transpose_512(nat, f"w{iw}T{es}")
            ps = bank_tile()
            if mode == "nat":
                for dt in range(NT):
                    nc.tensor.matmul(
                        ps,
                        lhsT=xT[:, dt, :],
                        rhs=wT[:, :, dt, :],
                        start=(dt == 0),
                        stop=(dt == NT - 1),
                    )
                copy_cast(vn[:, es * 512 : (es + 1) * 512], ps)
            else:
                # direct transposed projection: [e-chunk, s] slices
                dest = qT if mode == "qT" else knT
                for c in range(4):
                    for dt in range(NT):
                        nc.tensor.matmul(
                            ps[:, c * 128 : (c + 1) * 128],
                            lhsT=wT[:, c, dt, :],
                            rhs=xT[:, dt, :],
                            start=(c == 0 and dt == 0),
                            stop=(c == 3 and dt == NT - 1),
                        )
                copy_cast(
                    dest[:, es * 4 : (es + 1) * 4, :].rearrange("p c s -> p (c s)"),
                    ps,
                )

    # ---------------- scores = q @ k^T ----------------
    scores = res.tile([128, KVT], F32, tag="scores", name=f"scores{R}")
    for ck in range(8):  # 8 chunks of 512 cache positions
        nat = load_cast_512(kc_d, ck * 512, f"kn{ck}")
        kT = transpose_512(nat, f"kT{ck}")
        ps = bank_tile()
        for et in range(NT):
            nc.tensor.matmul(
                ps,
                lhsT=qT[:, et, :],
                rhs=kT[:, :, et, :],
                start=(et == 0),
                stop=(et == NT - 1),
            )
        copy_cast(scores[:, ck * 512 : (ck + 1) * 512], ps)
    # new-token scores (kv = 4096..4223)
    ps = bank_tile()
    for et in range(NT):
        nc.tensor.matmul(
            ps[:, :128],
            lhsT=qT[:, et, :],
            rhs=knT[:, et, :],
            start=(et == 0),
            stop=(et == NT - 1),
        )
    copy_cast(scores[:, KV:KVT], ps[:, :128])

    # ---------------- softmax ----------------
    negm = small.tile([128, 1], F32, tag="negm", name=f"negm{R}")
    nc.vector.reduce_max(out=negm, in_=scores, axis=AX, negate=True)
    negms = small.tile([128, 1], F32, tag="negms", name=f"negms{R}")
    nc.vector.tensor_scalar_mul(negms, negm, SCALE)  # -max * scale

    attnT = res.tile([128, NKVT, 128], BF16, tag="attnT", name=f"attnT{R}")
    lparts = small.tile([128, 9], F32, tag="lparts", name=f"lparts{R}")
    for ck in range(9):
        w = 512 if ck < 8 else 128
        ac = achunk.tile([128, 512], BF16, tag="ac", name=f"ac{ck}{R}")
        nc.scalar.activation(
            out=ac[:, :w],
            in_=scores[:, ck * 512 : ck * 512 + w],
            func=Exp,
            bias=negms,
            scale=SCALE,
            accum_out=lparts[:, ck : ck + 1],
        )
        pe_transpose_block(
            [ac[:, t * 128 : (t + 1) * 128] for t in range(w // 128)],
            lambda b0, nb, ck=ck: attnT[
                :, ck * 4 + b0 : ck * 4 + b0 + nb, :
            ].rearrange("p t s -> p (t s)"),
        )
    lsum = small.tile([128, 1], F32, tag="lsum", name=f"lsum{R}")
    nc.vector.reduce_sum(out=lsum, in_=lparts, axis=AX)
    rinv = small.tile([128, 1], F32, tag="rinv", name=f"rinv{R}")
    nc.vector.reciprocal(out=rinv, in_=lsum)

    # ------------- out = attn @ v  (natural, unnormalized) -------
    ps_out = []
    for _i in range(4):
        ps_out.append(
            bankp.tile([128, 512], F32, tag="bank", name=f"psout{_i}{R}")
        )
    for ck in range(8):  # v cache chunks of 512 rows
        vt = load_cast_512(vc_d, ck * 512, f"vld{ck}")
        for c in range(4):
            kvt = ck * 4 + c
            for ec in range(4):
                nc.tensor.matmul(
                    ps_out[ec],
                    lhsT=attnT[:, kvt, :],
                    rhs=vt[:, c, ec * 512 : (ec + 1) * 512],
                    start=(kvt == 0),
                    stop=False,
                )
    # v_new tail
    for ec in range(4):
        nc.tensor.matmul(
            ps_out[ec],
            lhsT=attnT[:, NKVT - 1, :],
            rhs=vn[:, ec * 512 : (ec + 1) * 512],
            start=False,
            stop=True,
        )
    out_bf = res.tile([128, D], BF16, tag="out_bf", name=f"out_bf{R}")
    for ec in range(4):
        copy_cast(out_bf[:, ec * 512 : (ec + 1) * 512], ps_out[ec])
    outT = res.tile([128, NT, 128], BF16, tag="outT", name=f"outT{R}")
    pe_transpose_block(
        [out_bf[:, t * 128 : (t + 1) * 128] for t in range(NT)],
        lambda b0, nb: outT[:, b0 : b0 + nb, :].rearrange("p t s -> p (t s)"),
    )

    # ---------------- y = out @ wo^T, row-scaled by 1/l ----------
    y_sb = res.tile([128, D], F32, tag="ysb", name=f"ysb{R}")
    for ds_ in range(4):
        nat = load_cast_512(wo_d, ds_ * 512, f"won{ds_}")
        woT = transpose_512(nat, f"woT{ds_}")
        ps = bank_tile()
        for et in range(NT):
            nc.tensor.matmul(
                ps,
                lhsT=outT[:, et, :],
                rhs=woT[:, :, et, :],
                start=(et == 0),
                stop=(et == NT - 1),
            )
        nc.scalar.activation(
            out=y_sb[:, ds_ * 512 : (ds_ + 1) * 512],
            in_=ps,
            func=Copy,
            scale=rinv,
        )
        nc.sync.dma_start(
            out=y_d[:, ds_ * 512 : (ds_ + 1) * 512],
            in_=y_sb[:, ds_ * 512 : (ds_ + 1) * 512],
        )


def build_nc(repeat: int = 1) -> bass.Bass:
    nc = bass.Bass()

    x_d = nc.declare_dram_parameter("x", [Q, D], F32, isOutput=False)
    kc_d = nc.declare_dram_parameter("k_cache", [KV, D], F32, isOutput=False)
    vc_d = nc.declare_dram_parameter("v_cache", [KV, D], F32, isOutput=False)
    wq_d = nc.declare_dram_parameter("wq", [D, D], F32, isOutput=False)
    wk_d = nc.declare_dram_parameter("wk", [D, D], F32, isOutput=False)
    wv_d = nc.declare_dram_parameter("wv", [D, D], F32, isOutput=False)
    wo_d = nc.declare_dram_parameter("wo", [D, D], F32, isOutput=False)
    y_d = nc.declare_dram_parameter("out", [Q, D], F32, isOutput=True)
    dram = (x_d, kc_d, vc_d, wq_d, wk_d, wv_d, wo_d, y_d)

    with tile.TileContext(nc) as tc:
        with (
            tc.tile_pool(name="res", bufs=1) as res,
            tc.tile_pool(name="nat", bufs=3) as natp,
            tc.tile_pool(name="T", bufs=2) as Tp,
            tc.tile_pool(name="achunk", bufs=3) as achunk,
            tc.tile_pool(name="small", bufs=2) as small,
            tc.tile_pool(name="bank", bufs=8, space="PSUM") as bankp,
        ):
            ident = res.tile([128, 128], BF16)
            make_identity(nc, ident)
            pools = (res, natp, Tp, achunk, small, bankp)
            state = {"flip": 0, "psn": 0}
            for rep in range(repeat):
                _emit_once(nc, pools, ident, state, rep, dram)

    return nc


# ---------------------------------------------------------------------------
# Host-side wrapper
# ---------------------------------------------------------------------------

_NC = None


def _get_nc():
    global _NC
    if _NC is None:
        _NC = build_nc()
    return _NC


def _make_in_maps(x, k_cache, v_cache, wq, wk, wv, wo):
    f = lambda a: np.ascontiguousarray(np.asarray(a), dtype=np.float32)
    x, k_cache, v_cache = f(x), f(k_cache), f(v_cache)
    wq, wk, wv, wo = f(wq), f(wk), f(wv), f(wo)
    maps = []
    for b in range(B):
        maps.append(
            {
                "x": x[b],
                "k_cache": k_cache[b],
                "v_cache": v_cache[b],
                "wq": wq,
                "wk": wk,
                "wv": wv,
                "wo": wo,
            }
        )
    return maps


def _install_hookfix():
    """The stock concourse neuronx_cc hook asserts a single HLO computation;
    this jax version lowers shard_map as `call`-indirected HLO. Patch the
    validation (operand order must still chain to entry params in order).
    Also split multi-wait/multi-update instructions in the BIR: this walrus
    build accepts at most one semaphore wait and one update per instruction."""
    import base64
    import tempfile

    import orjson

    from concourse import bass2jax
    from concourse.bass2jax import (
        _decompress_ant_bir,
        rename_neff_tensors_and_patch_header,
        compile_bir_kernel,
    )

    def _trace_to_param(id2ins, ins):
        seen = 0
        while ins.opcode != "parameter":
            if len(ins.operand_ids) != 1 or seen > 20:
                return None
            ins = id2ins[ins.operand_ids[0]]
            seen += 1
        return ins.parameter_number

    def _split_multi_waits(bir_bytes):
        """Move extra semaphore waits onto prepended single-wait
        EventSemaphore instructions on the same engine (engine streams are
        serial, so a preceding wait is equivalent); extra updates onto
        appended EventSemaphores."""
        bir = orjson.loads(bir_bytes)
        ctr = [0]
        for fn in bir.get("functions", []):
            for blk in fn.get("blocks", []):
                newins = []
                changed = False
                for ins in blk.get("instructions", []):
                    si = ins.get("sync_info")
                    eng = ins.get("engine")
                    waits = (si or {}).get("on_wait") or []
                    if si is not None and eng is not None and len(waits) > 1:
                        for w in waits[:-1]:
                            ctr[0] += 1
                            newins.append(
                                {
                                    "debug": ins.get("debug", 0),
                                    "engine": eng,
                                    "ins": [],
                                    "outs": [],
                                    "name": f"evw{ctr[0]}_{ins['name']}",
                                    "opcode": "EventSemaphore",
                                    "sync_info": {"on_update": [], "on_wait": [w]},
                                }
                            )
                        si["on_wait"] = [waits[-1]]
                        changed = True
                    newins.append(ins)
                    ups = (si or {}).get("on_update") or []
                    if si is not None and eng is not None and len(ups) > 1:
                        for u in ups[1:]:
                            ctr[0] += 1
                            newins.append(
                                {
                                    "debug": ins.get("debug", 0),
                                    "engine": eng,
                                    "ins": [],
                                    "outs": [],
                                    "name": f"evu{ctr[0]}_{ins['name']}",
                                    "opcode": "EventSemaphore",
                                    "sync_info": {"on_update": [u], "on_wait": []},
                                }
                            )
                        si["on_update"] = ups[:1]
                        changed = True
                if changed:
                    blk["instructions"] = newins
        return orjson.dumps(bir)

    def neuronx_cc_hook(code, code_format, platform_version, file_prefix):
        import libneuronxla
        import libneuronxla.proto.hlo_pb2
        from libneuronxla.libncc import _wrap_neff_as_custom_call

        assert libneuronxla.orig_neuronx_cc is not None

        def _orig():
            return libneuronxla.orig_neuronx_cc(
                code, code_format, platform_version, file_prefix
            )

        if b"bass_exec" not in code:
            return _orig()
        assert code_format.decode() == "hlo"
        code_proto = libneuronxla.proto.hlo_pb2.HloModuleProto.FromString(code)
        bass_exec_call = None
        body = None
        for comp in code_proto.computations:
            for ins in comp.instructions:
                if (
                    ins.opcode == "custom-call"
                    and ins.custom_call_target == "bass_exec"
                ):
                    assert bass_exec_call is None
                    bass_exec_call = ins
                    body = comp
        if bass_exec_call is None:
            return _orig()

        id2ins = {i.id: i for i in body.instructions}
        nums = [
            _trace_to_param(id2ins, id2ins[op_id])
            for op_id in bass_exec_call.operand_ids[:-1]
        ]
        if nums != list(range(len(nums))):
            raise ValueError(f"bass_exec operands not in param order: {nums}")

        entry = [
            c
            for c in code_proto.computations
            if c.id == code_proto.entry_computation_id
        ]
        entry = entry[0] if entry else code_proto.computations[-1]
        if entry.id != body.id:
            eid2ins = {i.id: i for i in entry.instructions}
            call_ins = [
                i
                for i in entry.instructions
                if i.opcode == "call"
                and list(i.called_computation_ids) == [body.id]
            ]
            assert len(call_ins) == 1
            enums = [
                _trace_to_param(eid2ins, eid2ins[op_id])
                for op_id in call_ins[0].operand_ids
            ]
            if enums != list(range(len(enums))):
                raise ValueError(f"shmap call operands not in param order: {enums}")

        config = orjson.loads(base64.standard_b64decode(bass_exec_call.backend_config))
        assert len(config["in_names"]) == len(bass_exec_call.operand_ids)
        in_rename = {name: f"input{i}" for i, name in enumerate(config["in_names"])}
        out_rename = {name: f"output{i}" for i, name in enumerate(config["out_names"])}
        neff_name = f"model_{code_proto.name.replace('/', '_')}.neff"
        ant_bir_str = _decompress_ant_bir(config["ant_bir"])
        ant_bir_str = _split_multi_waits(ant_bir_str)
        compile_dir = tempfile.TemporaryDirectory(delete=False)
        with compile_dir as compile_dir_path:
            neff_file = compile_bir_kernel(
                ant_bir_str, compile_dir_path, neff_name=neff_name
            )
            neff_data = rename_neff_tensors_and_patch_header(
                neff_file, in_rename | out_rename
            )
        compile_dir.cleanup()
        return 0, _wrap_neff_as_custom_call(code, neff_data)

    bass2jax.neuronx_cc_hook = neuronx_cc_hook
    try:
        import libneuronxla
    except ImportError:
        return
    if not hasattr(libneuronxla, "orig_neuronx_cc"):
        libneuronxla.orig_neuronx_cc = libneuronxla.neuronx_cc
    libneuronxla.neuronx_cc = neuronx_cc_hook


def run(inputs, trace=False, trace_cores=None):
    _install_hookfix()
    nc = _get_nc()
    maps = _make_in_maps(
        inputs["x"], inputs["k_cache"], inputs["v_cache"],
        inputs["wq"], inputs["wk"], inputs["wv"], inputs["wo"],
    )
    res = run_bass_kernel_spmd(
        nc,
        maps,
        core_ids=list(range(B)),
        trace=trace,
        trace_cores=trace_cores,
    )
    y = np.stack([res.results[b]["out"] for b in range(B)], axis=0)
    return y, res


def kernel(x, mask, k_cache, v_cache, wq, wk, wv, wo):
    y, _ = run(
        {
            "x": x,
            "k_cache": k_cache,
            "v_cache": v_cache,
            "wq": wq,
            "wk": wk,
            "wv": wv,
            "wo": wo,
        }
    )
    return y
